# revision 90
# baseline (speedup 1.0000x reference)
"""Trainium2 Bass kernel for nn_NoiseFilter.

Math (negacyclic-transform direct complex product, validated to 2e-14 in f64):
per frame (noise u[256], amp[65]):
    x  = 2u - 1                      (folded into the host-side fp16 cast)
    X' = x @ A        # [512] = (Re | Im) of 256 odd-frequency (negacyclic) bins
    H' = amp @ B      # [512]
    p  = XR*HR - XI*HI               # Re(X'H')  [256]
    q  = XR*HI + XI*HR               # Im(X'H')  [256]
    out = [p|q] @ E   # [256]  negacyclic inverse == linear conv (support 511)

The odd-frequency (negacyclic) DFT has no degenerate real bins: exactly 256
generic complex bins = 512 real slots, so the complex product is 4 bulk
multiplies + 2 bulk add/subs with no special-cased slots.

On-chip dataflow per 512-frame block (inputs host-pre-transposed to
[coeff, frame] layout; noise pre-split to fp8 hi+lo planes on host —
same bytes as fp16 — so the forward transform runs fp8 DoubleRow):
    xt   [128,2(kt),2(hi/lo),512] fp8 <- DMA   (time-major noise)
    at   [65,512] fp16                <- DMA   (amp)
    H'   = B-chunks @ at (fp16)       -> PSUM -> ACT copy -> h_sb fp16
    X'   = 3 DoubleRow fp8 matmuls/chunk (Ah.xh + Ah.xl + Al.xh, each DR
           packs both 128-row time k-tiles at 0.5 cyc/row) -> PSUM -> ACT
    t1..t4, p, q  on DVE (fp16 SBUF, 2x mode)     -> m_sb fp16
    out  = sum_c E[c]^T @ m[c]  -> PSUM -> DVE copy -> DMA (fp16, [t, frame])
Host transposes the [256, frames] fp16 result back and casts fp32.

The fp8 hi/lo error-split keeps the forward exact to ~1.4e-3 rel (vs the
2e-2 gate): x and A are each represented as e4m3 value + e4m3 residual,
and the Al.xl term (~1e-4 rel) is dropped.  GPSIMD/Pool cannot touch
PSUM on real HW, so all PSUM->SBUF copies stay on ACT/DVE exactly as in
the fp16 schedule (which saturates ACT~4.2us / DVE~4.4us per block);
the fp8 forward cuts PE from 4443 to 3840 ns/block, taking PE off the
critical path.  The inverse of block n-2 is interleaved into block n's
forward matmuls; warmup matmuls cover the p-state ramp + DMA startup.
Data parallel over 8 cores: 8192 frames/core.
"""

import os

import numpy as np

os.environ.setdefault("MYCRO_LOCAL_CACHE", "1")

HOP = 256
NB = 65
B_DIM = 16
F_DIM = 4096
NCORES = 8
FRAMES = B_DIM * F_DIM
FR_PER_CORE = FRAMES // NCORES  # 8192
BLK = 512                        # frames per block


# ---------------------------------------------------------------- matrices
def _e4m3():
    import ml_dtypes
    return (ml_dtypes.float8_e4m3fn if hasattr(ml_dtypes, "float8_e4m3fn")
            else ml_dtypes.float8_e4m3)


def _split8(x):
    E4 = _e4m3()
    hi = np.asarray(x, np.float32).astype(E4)
    lo = (np.asarray(x, np.float64) - hi.astype(np.float64)).astype(np.float32).astype(E4)
    return hi, lo


def _build_matrices():
    FS = 128
    N = 512
    t = np.arange(HOP)
    s = np.arange(256)
    # negacyclic (odd-frequency) analysis: X'[s] = sum_t x[t] e^{-2pi i t (s+1/2)/N}
    W = np.exp(-2j * np.pi * np.outer(t, s + 0.5) / N)   # [256, 256]
    A = np.concatenate([W.real, W.imag], axis=1)          # [256, 512]

    eye = np.eye(NB)
    ir = np.fft.irfft(eye, axis=-1)                       # [65, 128]
    ir = np.roll(ir, FS // 2, axis=-1)
    n = np.arange(FS)
    win = 0.5 * (1.0 - np.cos(2.0 * np.pi * n / FS))
    ir = ir * win
    ir = np.pad(ir, ((0, 0), (0, HOP - FS)))
    M_imp = np.roll(ir, -(FS // 2), axis=-1)              # [65, 256]

    D = M_imp @ W                                         # [65, 256] complex
    Bm = np.concatenate([D.real, D.imag], axis=1)         # [65, 512]

    # inverse: out[n] = (2/N) Re sum_s Y'[s] e^{+2pi i n (s+1/2)/N}
    nn = np.arange(HOP)
    Winv = np.exp(2j * np.pi * np.outer(s + 0.5, nn) / N)  # [256, 256]
    E = np.concatenate([(2.0 / N) * Winv.real,             # multiplies p
                        -(2.0 / N) * Winv.imag], axis=0)   # multiplies q

    Ah, Al = _split8(A)                                    # [256, 512] fp8
    return (Ah, Al,
            np.ascontiguousarray(Bm, np.float16),
            np.ascontiguousarray(E, np.float16))


# ---------------------------------------------------------------- bass kernel
def _emit_kernel(ctx, tc, xt_d, amp_d, ah_cst, al_cst, b_cst, e_cst, out_d,
                 n_frames):
    import concourse.mybir as mybir

    nc = tc.nc
    f32 = mybir.dt.float32
    f16 = mybir.dt.float16
    f8 = mybir.dt.float8e4
    DR = mybir.MatmulPerfMode.DoubleRow
    Copy = mybir.ActivationFunctionType.Copy
    mult = mybir.AluOpType.mult
    add = mybir.AluOpType.add
    sub = mybir.AluOpType.subtract

    assert n_frames % BLK == 0
    nfull = n_frames // BLK
    sizes = [BLK] * nfull
    starts = [sum(sizes[:i]) for i in range(len(sizes))]
    nblk = len(sizes)
    PIPE = 2  # inverse trails the forward pass by 2 blocks

    singles = ctx.enter_context(tc.tile_pool(name="singles", bufs=1))
    p_xt = ctx.enter_context(tc.tile_pool(name="p_xt", bufs=3))
    p_at = ctx.enter_context(tc.tile_pool(name="p_at", bufs=3))
    p_h = ctx.enter_context(tc.tile_pool(name="p_h", bufs=3))
    p_x = ctx.enter_context(tc.tile_pool(name="p_x", bufs=3))
    p_t = ctx.enter_context(tc.tile_pool(name="p_t", bufs=2))
    p_m = ctx.enter_context(tc.tile_pool(name="p_m", bufs=PIPE + 2))
    p_o = ctx.enter_context(tc.tile_pool(name="p_o", bufs=3))
    ps_b = ctx.enter_context(tc.tile_pool(name="ps_b", bufs=1, space="PSUM"))
    ps_a = ctx.enter_context(tc.tile_pool(name="ps_a", bufs=2, space="PSUM"))
    ps_o = ctx.enter_context(tc.tile_pool(name="ps_o", bufs=1, space="PSUM"))

    # constants (big ones via SWDGE so the HWDGE queue serves block-0 inputs)
    b_sb = singles.tile([NB, 4, 128], f16)
    nc.sync.dma_start(out=b_sb, in_=b_cst.rearrange("k (c s) -> k c s", s=128))
    e_sb = singles.tile([128, 4, 2, 128], f16)
    ah_sb = singles.tile([128, 2, 4, 128], f8)
    nc.gpsimd.dma_start(out=ah_sb, in_=ah_cst.rearrange(
        "(kt p) (c s) -> p kt c s", p=128, s=128))
    al_sb = singles.tile([128, 2, 4, 128], f8)
    nc.gpsimd.dma_start(out=al_sb, in_=al_cst.rearrange(
        "(kt p) (c s) -> p kt c s", p=128, s=128))

    xv = xt_d.rearrange("(p kt pl) F -> p kt pl F", kt=2, pl=2)
    av = amp_d
    ov = out_d.rearrange("(jt p) F -> p jt F", p=128)

    # PE warmup: keep PE busy through DMA startup; 4 matmuls suffice since
    # block-0's own (mid-clock) matmuls finish the p-state ramp.
    warm = singles.tile([128, BLK], f16)
    nc.vector.memset(warm, 0.0)
    for w in range(4):
        pw = ps_a.tile([128, 2, BLK], f32, tag="pa")
        nc.tensor.matmul(pw[:, 0, :], warm[:, 0:128], warm,
                         start=True, stop=True)

    m_ring = {}

    for b in range(nblk + PIPE):
        if b < nblk:
            sz = sizes[b]
            lo = starts[b]
            # ---- loads (already fp16, already coeff-major)
            at = p_at.tile([NB, BLK], f16, tag="at")
            nc.sync.dma_start(out=at[:, :sz], in_=av[:, lo:lo + sz])
            xt = p_xt.tile([128, 2, 2, BLK], f8, tag="xt")
            nc.sync.dma_start(out=xt[:, :, :, :sz], in_=xv[:, :, :, lo:lo + sz])
            if b == 0:
                nc.gpsimd.dma_start(out=e_sb, in_=e_cst.rearrange(
                    "(c p) (j t) -> p c j t", p=128, t=128))

            # ---- H' = amp @ B, X' = x @ A  (PE), ACT copies to fp16
            h_sb = p_h.tile([128, 4, BLK], f16, tag="h")
            x_sb = p_x.tile([128, 4, BLK], f16, tag="x")
            pb = {}
            pa = {}
            po = mo = None
            o_sb = None
            osz = olo = 0
            if b >= PIPE:
                osz = sizes[b - PIPE]
                olo = starts[b - PIPE]
                mo = m_ring.pop(b - PIPE)
                po_0 = ps_o.tile([128, BLK], f32, tag="po0")
                po_1 = ps_o.tile([128, BLK], f32, tag="po1")
                po = [po_0, po_1]
                o_sb = p_o.tile([128, 2, BLK], f16, tag="o")

            def emit_inv(j, _po=po, _mo=mo, _osz=osz, _olo=olo, _o=o_sb, _b=b):
                for c in range(4):
                    nc.tensor.matmul(_po[j][:, :_osz], e_sb[:, c, j, :],
                                     _mo[:, c, :_osz],
                                     start=(c == 0), stop=(c == 3))
                if _b >= nblk - 1:
                    nc.scalar.activation(out=_o[:, j, :_osz],
                                         in_=_po[j][:, :_osz], func=Copy)
                else:
                    nc.vector.tensor_copy(_o[:, j, :_osz], _po[j][:, :_osz])
                nc.sync.dma_start(out=ov[:, j:j + 1, _olo:_olo + _osz],
                                  in_=_o[:, j:j + 1, :_osz])
            for half in range(2):
                pb_t = ps_b.tile([128, 2, BLK], f32, tag="pb")
                pb[half] = pb_t
                for cc in range(2):
                    c = half * 2 + cc
                    nc.tensor.matmul(pb_t[:, cc, :sz], b_sb[:, c, :],
                                     at[:, :sz], start=True, stop=True)
                pa_t = ps_a.tile([128, 2, BLK], f32, tag="pa")
                pa[half] = pa_t
                for cc in range(2):
                    c = half * 2 + cc
                    nc.tensor.matmul(pa_t[:, cc, :sz], ah_sb[:, :, c, :],
                                     xt[:, :, 0, :sz], start=True, stop=False,
                                     perf_mode=DR)
                    nc.tensor.matmul(pa_t[:, cc, :sz], ah_sb[:, :, c, :],
                                     xt[:, :, 1, :sz], start=False, stop=False,
                                     perf_mode=DR)
                    nc.tensor.matmul(pa_t[:, cc, :sz], al_sb[:, :, c, :],
                                     xt[:, :, 0, :sz], start=False, stop=True,
                                     perf_mode=DR)
                # copies for this half (ACT), x first (feeds RR)
                nc.scalar.activation(out=x_sb[:, half * 2:half * 2 + 2, :sz],
                                     in_=pa_t[:, :, :sz], func=Copy)
                nc.scalar.activation(out=h_sb[:, half * 2:half * 2 + 2, :sz],
                                     in_=pb_t[:, :, :sz], func=Copy)
                # interleaved inverse t-tile j of block b-PIPE, then its
                # copy + DMA immediately (independent PSUM tile per j)
                if po is not None and b < nblk - 1:
                    emit_inv(half)
                elif b < PIPE:
                    for _w in range(1):
                        pw = ps_a.tile([128, 2, BLK], f32, tag="pa")
                        nc.tensor.matmul(pw[:, 0, :], warm[:, 0:128], warm,
                                         start=True, stop=True)
            if po is not None and b == nblk - 1:
                emit_inv(0)
                emit_inv(1)

            # ---- complex product (DVE fp16 2x): slots [XR|XI] x [HR|HI]
            t_sb = p_t.tile([128, 8, BLK], f16, tag="t")
            m_sb = p_m.tile([128, 4, BLK], f16, tag="m")
            nc.vector.tensor_tensor(out=t_sb[:, 0:2, :sz], in0=x_sb[:, 0:2, :sz],
                                    in1=h_sb[:, 0:2, :sz], op=mult)  # RR
            if b >= nblk - 1:  # II early so p (and the inverse's p-chunks) start sooner
                nc.vector.tensor_tensor(out=t_sb[:, 2:4, :sz], in0=x_sb[:, 2:4, :sz],
                                        in1=h_sb[:, 2:4, :sz], op=mult)  # II
                nc.vector.tensor_tensor(out=m_sb[:, 0:2, :sz], in0=t_sb[:, 0:2, :sz],
                                        in1=t_sb[:, 2:4, :sz], op=sub)   # p
            nc.vector.tensor_tensor(out=t_sb[:, 4:6, :sz], in0=x_sb[:, 0:2, :sz],
                                    in1=h_sb[:, 2:4, :sz], op=mult)  # RI
            nc.vector.tensor_tensor(out=t_sb[:, 6:8, :sz], in0=x_sb[:, 2:4, :sz],
                                    in1=h_sb[:, 0:2, :sz], op=mult)  # IR
            if b < nblk - 1:
                nc.vector.tensor_tensor(out=t_sb[:, 2:4, :sz], in0=x_sb[:, 2:4, :sz],
                                        in1=h_sb[:, 2:4, :sz], op=mult)  # II
            q_eng = nc.vector if b >= nblk - 1 else nc.gpsimd
            q_eng.tensor_tensor(out=m_sb[:, 2:4, :sz],
                                in0=t_sb[:, 4:6, :sz],
                                in1=t_sb[:, 6:8, :sz], op=add)   # q
            if b < nblk - 1:
                nc.vector.tensor_tensor(out=m_sb[:, 0:2, :sz],
                                        in0=t_sb[:, 0:2, :sz],
                                        in1=t_sb[:, 2:4, :sz], op=sub)   # p
            m_ring[b] = m_sb

            if b == nblk - 1 and b - 1 >= PIPE - 1 and (b - 1) in m_ring:
                # pull the second-to-last inverse into this block so only one
                # inverse remains in the drain
                osz2 = sizes[b - 1]
                olo2 = starts[b - 1]
                mo2 = m_ring.pop(b - 1)
                o_sb2 = p_o.tile([128, 2, BLK], f16, tag="o")
                for j in range(2):
                    po_j = ps_o.tile([128, BLK], f32, tag=f"po{j}")
                    for c in range(4):
                        nc.tensor.matmul(po_j[:, :osz2], e_sb[:, c, j, :],
                                         mo2[:, c, :osz2],
                                         start=(c == 0), stop=(c == 3))
                    nc.scalar.activation(out=o_sb2[:, j, :osz2],
                                         in_=po_j[:, :osz2], func=Copy)
                    nc.sync.dma_start(out=ov[:, j:j + 1, olo2:olo2 + osz2],
                                      in_=o_sb2[:, j:j + 1, :osz2])
        elif b >= PIPE and (b - PIPE) in m_ring:
            osz = sizes[b - PIPE]
            olo = starts[b - PIPE]
            mo = m_ring.pop(b - PIPE)
            o_sb = p_o.tile([128, 2, BLK], f16, tag="o")
            po_d = []
            for j in range(2):
                po_j = ps_o.tile([128, BLK], f32, tag=f"po{j}")
                po_d.append(po_j)
                for c in range(4):
                    nc.tensor.matmul(po_j[:, :osz], e_sb[:, c, j, :],
                                     mo[:, c, :osz],
                                     start=(c == 0), stop=(c == 3))
                nc.scalar.activation(out=o_sb[:, j, :osz], in_=po_j[:, :osz],
                                     func=Copy)
                # j0 via SWDGE so the final j1 HWDGE gen isn't queued behind it
                eng = nc.gpsimd if j == 0 else nc.sync
                eng.dma_start(out=ov[:, j:j + 1, olo:olo + osz],
                              in_=o_sb[:, j:j + 1, :osz])




def build_nc(n_frames=FR_PER_CORE):
    import concourse.bacc as bacc
    import concourse.mybir as mybir
    import concourse.tile as tile

    f16 = mybir.dt.float16
    f32 = mybir.dt.float32
    nc = bacc.Bacc("TRN2", target_bir_lowering=False, debug=False)
    f8 = mybir.dt.float8e4
    xt_d = nc.dram_tensor("xt", [2 * HOP, n_frames], f8, kind="ExternalInput").ap()
    amp_d = nc.dram_tensor("ampt", [NB, n_frames], f16, kind="ExternalInput").ap()
    ah_cst = nc.dram_tensor("ah_cst", [HOP, 512], f8, kind="ExternalInput").ap()
    al_cst = nc.dram_tensor("al_cst", [HOP, 512], f8, kind="ExternalInput").ap()
    b_cst = nc.dram_tensor("b_cst", [NB, 512], f16, kind="ExternalInput").ap()
    e_cst = nc.dram_tensor("e_cst", [512, HOP], f16, kind="ExternalInput").ap()
    out_d = nc.dram_tensor("out", [HOP, n_frames], f16, kind="ExternalOutput").ap()

    from contextlib import ExitStack

    with tile.TileContext(nc) as tc, ExitStack() as ctx:
        _emit_kernel(ctx, tc, xt_d, amp_d, ah_cst, al_cst, b_cst, e_cst, out_d,
                     n_frames)
    nc.compile()
    return nc


_CACHE = {}


def _get(n_frames=FR_PER_CORE):
    key = n_frames
    if key not in _CACHE:
        _CACHE[key] = (build_nc(n_frames), _build_matrices())
    return _CACHE[key]


def run_sharded(noise_flat, amp_flat, n_frames_per_core, n_cores, trace=False):
    """noise_flat: [n, 256] fp32 u-noise; amp_flat: [n, 65] fp32."""
    from concourse import bass_utils

    nc, (Ah, Al, Bm, E) = _get(n_frames_per_core)
    x = 2.0 * noise_flat.astype(np.float32) - 1.0
    xh, xl = _split8(x)
    nf = x.shape[0]
    xs = np.empty((128, 2, 2, nf), dtype=xh.dtype)
    xhT = xh.T.reshape(2, 128, nf)
    xlT = xl.T.reshape(2, 128, nf)
    xs[:, 0, 0], xs[:, 1, 0] = xhT[0], xhT[1]
    xs[:, 0, 1], xs[:, 1, 1] = xlT[0], xlT[1]
    xs = xs.reshape(512, nf)
    a16 = amp_flat.astype(np.float16)
    in_maps = []
    for i in range(n_cores):
        lo, hi = i * n_frames_per_core, (i + 1) * n_frames_per_core
        in_maps.append({
            "xt": np.ascontiguousarray(xs[:, lo:hi]),
            "ampt": np.ascontiguousarray(a16[lo:hi].T),
            "ah_cst": Ah, "al_cst": Al, "b_cst": Bm, "e_cst": E,
        })
    res = bass_utils.run_bass_kernel_spmd(
        nc, in_maps, core_ids=list(range(n_cores)), trace=trace
    )
    out = np.concatenate(
        [res.results[i]["out"].T for i in range(n_cores)], axis=0)
    return out.astype(np.float32), res


def kernel(filter_bank, noise_u):
    fb = np.asarray(filter_bank, np.float32).reshape(-1, NB)
    nu = np.asarray(noise_u, np.float32).reshape(-1, HOP)
    out, _ = run_sharded(nu, fb, FR_PER_CORE, NCORES)
    return out.reshape(B_DIM, F_DIM * HOP, 1).astype(np.float32)


if __name__ == "__main__":
    nc = build_nc(BLK * 2)
    print("built OK")

